# revision 37
# baseline (speedup 1.0000x reference)
"""Trainium2 Bass kernel for the attention-score MLP module.

Reference computation (B=16, L=2048, E=D=1024):
    att1 = encoder_out @ W_enc + b_enc            # [B, L, D]
    att2 = decoder_hidden @ W_dec + b_dec         # [B, D]
    att  = relu(att1 + att2[:, None, :]) @ w_full + b_full   # [B, L]
    alpha = softmax(att, axis=1)                  # [B, L]
    context = einsum("ble,bl->be", encoder_out, alpha)
    returns (context, alpha)

Sharding: data-parallel over batch across 8 NeuronCores (2 batches/core),
weights replicated.

Per-core kernel layout notes:
  - The big matmul runs in "out = [d, l]" orientation: lhsT = W_enc chunk
    (natural [e, d] layout), rhs = enc.T chunks (PE-transposed on the fly).
    That makes att2 a per-partition bias, so the PSUM->SBUF eviction fuses
    (add bias + relu) into one scalar-engine activation.
  - b_full is skipped: softmax is invariant to a uniform shift.
  - Softmax is blockwise-stable (flash-attention style): each 512-l block
    subtracts its own max before exp; blocks are combined exactly in the
    batch epilogue, and the streamed context accumulator is online-rescaled
    by exp(m_old - m_new). encoder_out is read exactly once and the kernel
    is safe at any input scale.
  - Matmuls use float32r (full-rate fp32 mode, ~tf32 precision).
  - Weight/enc DMA issue order is tuned so compute starts ~6us in.
"""

from contextlib import ExitStack

import numpy as np

import concourse.bass as bass
import concourse.mybir as mybir
import concourse.tile as tile
from concourse import bass_utils

AF = mybir.ActivationFunctionType
F32 = mybir.dt.float32
F32R = mybir.dt.float32r

N_CORES = 8
B, L, E, D = 16, 2048, 1024, 1024
BL = B // N_CORES           # batches per core
P = 128
LB = 512                    # l-block size
NLB = L // LB               # l-blocks per batch
NLC = LB // P               # 128-row l-chunks per l-block
EC = E // P                 # e-chunks
DC = D // P                 # d-chunks

# dtype for the main/score/context matmul operands
# (float32r = full-rate fp32 mode, ~tf32 precision; F32 = exact, 1/4 rate)
MM_DT = F32R


def _split_multi_waits(nc, max_waits=1):
    """The walrus build in this container rejects instructions carrying more
    than one sync wait. Move excess waits onto standalone EventSemaphore
    instructions inserted just before, on the same engine stream."""
    import bass_rust

    counter = 0
    for func in nc.m.functions:
        for blk in func.blocks:
            insts = blk.instructions
            if not any(
                getattr(i, "sync_info", None) is not None
                and len(i.sync_info.on_wait) > max_waits
                for i in insts
            ):
                continue
            new_list = []
            for inst in insts:
                si = getattr(inst, "sync_info", None)
                if si is not None:
                    waits = list(si.on_wait)
                    if len(waits) > max_waits:
                        for w in waits[:-max_waits]:
                            es = mybir.InstEventSemaphore(
                                name=f"WSPLIT-{counter}", ins=[], outs=[]
                            )
                            counter += 1
                            es.engine = inst.engine
                            es.sync_info = bass_rust.SyncInfo(
                                on_wait=[w], on_update=[]
                            )
                            new_list.append(es)
                        si.on_wait = waits[-max_waits:]
                        inst.sync_info = si
                new_list.append(inst)
            blk.instructions = new_list
    return counter


def _build_body(nc, tc, ctx, reps=1):
    enc_d = nc.dram_tensor("enc", [BL, L, E], F32, kind="ExternalInput").ap()
    dec_d = nc.dram_tensor("dec", [BL, D], F32, kind="ExternalInput").ap()
    wenc_d = nc.dram_tensor("wenc", [E, D], F32, kind="ExternalInput").ap()
    wdec_d = nc.dram_tensor("wdec", [D, D], F32, kind="ExternalInput").ap()
    benc_d = nc.dram_tensor("benc", [D], F32, kind="ExternalInput").ap()
    bdec_d = nc.dram_tensor("bdec", [D], F32, kind="ExternalInput").ap()
    wfull_d = nc.dram_tensor("wfull", [D], F32, kind="ExternalInput").ap()
    ident_d = nc.dram_tensor("ident", [P, P], F32, kind="ExternalInput").ap()
    ctx_d = nc.dram_tensor("context", [BL, E], F32, kind="ExternalOutput").ap()
    alpha_d = nc.dram_tensor("alpha", [BL, L], F32, kind="ExternalOutput").ap()

    def pool(name, bufs, space="SBUF"):
        return ctx.enter_context(tc.tile_pool(name=name, bufs=bufs, space=space))

    constp = pool("const", 1)
    wencp = pool("wenc", 1)
    wdecp = pool("wdec", 2)
    encp = pool("enc", 3)
    encTp = pool("encT", 2)
    relup = pool("relu", 2)
    exprp = pool("exprow", BL)
    ctxaccp = pool("ctxacc", BL)
    expTp = pool("expT", 2)
    smxp = pool("smx", BL)
    psTp = pool("psT", 2, "PSUM")
    psMp = pool("psM", 4, "PSUM")
    psCp = pool("psC", 1, "PSUM")
    psEp = pool("psE", 1, "PSUM")

    # ---- constants (small, on the SWDGE path to keep SP free for bulk DMA;
    # identity comes from DRAM so no slow gpsimd affine_select blocks the
    # Pool DMA issue stream)
    identr = constp.tile([P, P], MM_DT)
    nc.sync.dma_start(identr, ident_d.bitcast(MM_DT))
    ones_row = constp.tile([1, P], F32)
    nc.vector.memset(ones_row[:], 1.0)

    def load_enc_block(b, lb, parts=2):
        # split e-range DMAs so the first transposes start before the
        # whole block has landed
        et = encp.tile([P, NLC, E], MM_DT, tag="enc")
        src = enc_d[b, lb * LB : (lb + 1) * LB, :]
        w = E // parts
        for h in range(parts):
            nc.sync.dma_start(
                et[:, :, h * w : (h + 1) * w],
                src[:, h * w : (h + 1) * w]
                .rearrange("(i p) e -> p i e", p=P)
                .bitcast(MM_DT),
            )
        return et

    # ---- bulk-DMA issue order: enc block 0, W_enc chunks, W_dec slices,
    # enc block 1; steady-state enc prefetch follows.
    enc_q = [load_enc_block(0, 0, parts=4)]

    wfullT = constp.tile([P, DC], MM_DT)
    nc.sync.dma_start(
        wfullT, wfull_d.rearrange("(j p) -> p j", p=P).bitcast(MM_DT)
    )
    bencT = constp.tile([P, DC], F32)
    nc.sync.dma_start(bencT, benc_d.rearrange("(j p) -> p j", p=P))
    biasT = constp.tile([P, DC], F32)
    nc.sync.dma_start(biasT, bdec_d.rearrange("(j p) -> p j", p=P))
    nc.vector.tensor_add(out=biasT[:], in0=biasT[:], in1=bencT[:])

    # dec.T chunks: decT[p, k, b] = dec[b, k*P + p]
    decT = constp.tile([P, EC, BL], F32)
    for bb in range(BL):
        nc.sync.dma_start(
            decT[:, :, bb], dec_d[bb].rearrange("(k p) -> p k", p=P)
        )

    wenc_sb = wencp.tile([P, EC, D], MM_DT)
    for ej in range(EC):
        nc.sync.dma_start(
            wenc_sb[:, ej, :],
            wenc_d[ej * P : (ej + 1) * P, :]
            .rearrange("(o p) d -> p o d", p=P)
            .bitcast(MM_DT),
        )

    # att2.T (+ b_enc + b_dec): att2T[p, dj, b] per-partition bias, computed
    # from W_dec column slices as they stream in
    att2T = constp.tile([P, DC, BL], F32)
    for dj in range(DC):
        wds = wdecp.tile([P, EC, P], F32, tag="wdec")
        nc.sync.dma_start(
            wds,
            wdec_d[:, dj * P : (dj + 1) * P].rearrange("(k p) d -> p k d", p=P),
        )
        # share the psE slot (free outside the main-loop exp transposes)
        psf = psEp.tile([P, NLC], F32, tag="psE", name=f"psa2_{dj}")
        ps = psf[:, :BL]
        for k in range(EC):
            nc.tensor.matmul(
                ps[:], lhsT=wds[:, k, :], rhs=decT[:, k, :],
                start=(k == 0), stop=(k == EC - 1),
            )
        nc.scalar.activation(
            att2T[:, dj, :], ps[:], AF.Identity, bias=biasT[:, dj : dj + 1]
        )

    enc_q.append(load_enc_block(0, 1))

    flat = [(b, lb) for _ in range(reps) for b in range(BL) for lb in range(NLB)]

    def transpose_block(enc_blk):
        # j-major so each e-chunk of encT completes early (main matmul
        # group ej consumes chunks in the same order), one bulk eviction
        # per chunk, alternating DVE/ACT to balance engine load
        encT = encTp.tile([P, EC, LB], MM_DT, tag="encT")
        for j in range(EC):
            pst = psTp.tile([P, NLC * P], MM_DT, tag="psT")
            for i in range(NLC):
                nc.tensor.transpose(
                    pst[:, i * P : (i + 1) * P],
                    enc_blk[:, i, j * P : (j + 1) * P],
                    identr[:],
                )
            if j % 2 == 0:
                nc.vector.tensor_copy(out=encT[:, j, :], in_=pst[:])
            else:
                nc.scalar.copy(encT[:, j, :], pst[:])
        return encT

    def emit_main(g, enc_blk, encT, relu_out):
        b, lb = flat[g]
        for dj in range(DC):
            psm = psMp.tile([P, LB], F32, tag="psM", name=f"psm_{g}_{dj}")
            for ej in range(EC):
                nc.tensor.matmul(
                    psm[:],
                    lhsT=wenc_sb[:, ej, dj * P : (dj + 1) * P],
                    rhs=encT[:, ej, :],
                    start=(ej == 0),
                    stop=(ej == EC - 1),
                )
            # fused (+att2 bias, relu) eviction
            nc.scalar.activation(
                relu_out[:, dj, :], psm[:], AF.Relu,
                bias=att2T[:, dj, b : b + 1],
            )

    def emit_score_exp(g, relu, exp_row, zrow, mneg_row):
        b, lb = flat[g]
        pss = psEp.tile([1, LB], F32, tag="psE", name=f"pss_{g}")
        for dj in range(DC):
            nc.tensor.matmul(
                pss[:],
                lhsT=wfullT[:, dj : dj + 1],
                rhs=relu[:, dj, :],
                start=(dj == 0),
                stop=(dj == DC - 1),
            )
        # per-block max (negated) then exp(att - m_g); blocks are combined
        # exactly in the epilogue, so softmax is safe at any input scale
        nc.vector.reduce_max(
            out=mneg_row[:, lb : lb + 1], in_=pss[:],
            axis=mybir.AxisListType.X, negate=True,
        )
        nc.scalar.activation(
            exp_row[:, lb * LB : (lb + 1) * LB], pss[:], AF.Exp,
            bias=mneg_row[:, lb : lb + 1],
            accum_out=zrow[:, lb : lb + 1],
        )

    def emit_ctx(g, enc_blk, exp_row, ctx_acc, mrun, mneg_row):
        b, lb = flat[g]
        # online rescale state for the context accumulator:
        #   mrun = running (negated) max, s = exp(m_run_old - m_run_new),
        #   u = exp(m_g - m_run_new)
        if lb == 0:
            nc.vector.tensor_copy(out=mrun[:], in_=mneg_row[:, 0:1])
            sfac = None
            ufac = None
        else:
            newrun = smxp.tile([1, 1], F32, tag="newrun", name=f"newrun_{g}")
            nc.vector.tensor_tensor(
                out=newrun[:], in0=mrun[:], in1=mneg_row[:, lb : lb + 1],
                op=mybir.AluOpType.min,
            )
            sdif = smxp.tile([1, 1], F32, tag="sdif", name=f"sdif_{g}")
            nc.vector.tensor_sub(out=sdif[:], in0=newrun[:], in1=mrun[:])
            sfac = smxp.tile([1, 1], F32, tag="sfac", name=f"sfac_{g}")
            nc.scalar.activation(sfac[:], sdif[:], AF.Exp)
            udif = smxp.tile([1, 1], F32, tag="udif", name=f"udif_{g}")
            nc.vector.tensor_sub(
                out=udif[:], in0=newrun[:], in1=mneg_row[:, lb : lb + 1]
            )
            ufac = smxp.tile([1, 1], F32, tag="ufac", name=f"ufac_{g}")
            nc.scalar.activation(ufac[:], udif[:], AF.Exp)
            nc.vector.tensor_copy(out=mrun[:], in_=newrun[:])
            nc.vector.tensor_scalar_mul(
                out=ctx_acc[:], in0=ctx_acc[:], scalar1=sfac[:]
            )
        pse = psEp.tile([P, NLC], F32, tag="psE", name=f"pse_{g}")
        for i in range(NLC):
            c0 = lb * LB + i * P
            nc.tensor.transpose(
                pse[:, i : i + 1], exp_row[:, c0 : c0 + P],
                identr[0:1, 0:1].bitcast(F32),
            )
        expT = expTp.tile([P, NLC], MM_DT, tag="expT", name=f"expT_{g}")
        nc.vector.tensor_copy(out=expT[:], in_=pse[:])
        for half in range(2):
            psc = psCp.tile([1, 512], F32, tag="psC", name=f"psc_{g}_{half}")
            for i in range(NLC):
                nc.tensor.matmul(
                    psc[:],
                    lhsT=expT[:, i : i + 1],
                    rhs=enc_blk[:, i, half * 512 : (half + 1) * 512],
                    start=(i == 0),
                    stop=(i == NLC - 1),
                )
            dst = ctx_acc[:, half * 512 : (half + 1) * 512]
            if lb == 0:
                nc.vector.tensor_copy(out=dst, in_=psc[:])
            else:
                nc.vector.scalar_tensor_tensor(
                    out=dst, in0=psc[:], scalar=ufac[:], in1=dst,
                    op0=mybir.AluOpType.mult, op1=mybir.AluOpType.add,
                )

    def emit_epilogue(g, exp_row, ctx_acc, mrun, zrow, mneg_row):
        b, lb = flat[g]
        # ---- batch epilogue: combine per-block (m_g, z_g) states exactly:
        # M = max_g m_g,  v_g = exp(m_g - M) / sum_g exp(m_g - M) z_g;
        # ctx_acc is already at scale M via the online rescale
        urow = smxp.tile([1, NLB], F32, tag="urow", name=f"urow_{g}")
        # u_g = exp(m_g - M) = exp(-mneg_g + mrun)  (mrun = min_g mneg_g)
        nc.scalar.activation(
            urow[:], mneg_row[:], AF.Exp, bias=mrun[:], scale=-1.0
        )
        uz = smxp.tile([1, NLB], F32, tag="uz", name=f"uz_{g}")
        nc.vector.tensor_mul(out=uz[:], in0=urow[:], in1=zrow[:])
        zsum = smxp.tile([1, 1], F32, tag="zsum", name=f"zsum_{g}")
        nc.vector.reduce_sum(
            out=zsum[:], in_=uz[:], axis=mybir.AxisListType.X
        )
        rinv = smxp.tile([1, 1], F32, tag="rinv", name=f"rinv_{g}")
        nc.vector.reciprocal(rinv[:], zsum[:])
        vrow = smxp.tile([1, NLB], F32, tag="vrow", name=f"vrow_{g}")
        nc.vector.tensor_scalar_mul(out=vrow[:], in0=urow[:], scalar1=rinv[:])
        # alpha_g = exp_g * v_g ; context = sum_g v_g ctx_g
        for lb2 in range(NLB):
            nc.vector.tensor_scalar_mul(
                out=exp_row[:, lb2 * LB : (lb2 + 1) * LB],
                in0=exp_row[:, lb2 * LB : (lb2 + 1) * LB],
                scalar1=vrow[:, lb2 : lb2 + 1],
            )
        nc.gpsimd.dma_start(alpha_d[b : b + 1, :], exp_row[:])
        nc.vector.tensor_scalar_mul(
            out=ctx_acc[:], in0=ctx_acc[:], scalar1=rinv[:]
        )
        nc.gpsimd.dma_start(ctx_d[b : b + 1, :], ctx_acc[:])

    # software-pipelined emission: block g+1's transposes and main matmuls
    # are emitted between block g's score and context sections, so the PE
    # never idles through the softmax tail chain
    n_blocks = len(flat)
    rows = {}   # g -> (enc_blk, exp_row, ctx_acc)
    encTs = {}
    relus = {}

    def batch_tiles(g):
        b, lb = flat[g]
        if lb == 0:
            rows[g] = (
                exprp.tile([1, L], F32, tag="exp_row", name=f"exp_row_{g}"),
                ctxaccp.tile([1, E], F32, tag="ctx_acc", name=f"ctx_acc_{g}"),
                smxp.tile([1, 1], F32, tag="mrun", name=f"mrun_{g}"),
                smxp.tile([1, NLB], F32, tag="zrow", name=f"zrow_{g}"),
                smxp.tile([1, NLB], F32, tag="mneg", name=f"mneg_{g}"),
            )
        else:
            rows[g] = rows[g - 1]
        return rows[g]

    enc_blks = {0: enc_q.pop(0), 1: enc_q.pop(0)}

    def fetch(g):
        if g + 2 < n_blocks:
            nb, nlb = flat[g + 2]
            enc_blks[g + 2] = load_enc_block(nb, nlb)

    encTs[0] = transpose_block(enc_blks[0])
    relus[0] = relup.tile([P, DC, LB], MM_DT, tag="relu", name="relu_0")
    batch_tiles(0)
    fetch(0)
    emit_main(0, enc_blks[0], encTs[0], relus[0])

    for g in range(n_blocks):
        exp_row, ctx_acc, mrun, zrow, mneg_row = rows[g]
        if g + 1 < n_blocks:
            batch_tiles(g + 1)
            fetch(g + 1)
            encTs[g + 1] = transpose_block(enc_blks[g + 1])
        emit_score_exp(g, relus[g], exp_row, zrow, mneg_row)
        if g + 1 < n_blocks:
            relus[g + 1] = relup.tile(
                [P, DC, LB], MM_DT, tag="relu", name=f"relu_{g+1}"
            )
            emit_main(g + 1, enc_blks[g + 1], encTs[g + 1], relus[g + 1])
        emit_ctx(g, enc_blks[g], exp_row, ctx_acc, mrun, mneg_row)
        b, lb = flat[g]
        if lb == NLB - 1:
            emit_epilogue(g, exp_row, ctx_acc, mrun, zrow, mneg_row)
        del enc_blks[g], encTs[g], relus[g]


def build_program(reps=1):
    nc = bass.Bass("TRN2", num_devices=1, debug=False, enable_asserts=False)
    with tile.TileContext(nc) as tc:
        with ExitStack() as ctx:
            _build_body(nc, tc, ctx, reps=reps)
    _split_multi_waits(nc)
    return nc


_IDENT = np.eye(128, dtype=np.float32)

_NC = None
_RUNNER = None


def _get_nc():
    global _NC
    if _NC is None:
        _NC = build_program()
    return _NC


def _get_runner():
    """Build the shard_map-jitted executable once and reuse it across calls."""
    global _RUNNER
    if _RUNNER is not None:
        return _RUNNER
    import jax
    from jax.sharding import Mesh, PartitionSpec
    from jax.experimental.shard_map import shard_map
    from concourse import bass2jax
    from concourse.bass2jax import _bass_exec_p, install_neuronx_cc_hook

    nc = _get_nc()
    install_neuronx_cc_hook()
    partition_name = nc.partition_id_tensor.name if nc.partition_id_tensor else None

    in_names, out_names, out_avals, zero_outs = [], [], [], []
    for alloc in nc.m.functions[0].allocations:
        if not isinstance(alloc, mybir.MemoryLocationSet):
            continue
        name = alloc.memorylocations[0].name
        if alloc.kind == "ExternalInput":
            if name != partition_name:
                in_names.append(name)
        elif alloc.kind == "ExternalOutput":
            out_names.append(name)
            shape = tuple(alloc.tensor_shape)
            dtype = mybir.dt.np(alloc.dtype)
            out_avals.append(jax.core.ShapedArray(shape, dtype))
            zero_outs.append(np.zeros(shape, dtype))
    n_params = len(in_names)
    all_in_names = list(in_names) + out_names
    if partition_name is not None:
        all_in_names.append(partition_name)

    def _body(*args):
        operands = list(args)
        if partition_name is not None:
            operands.append(bass2jax.partition_id_tensor())
        outs = _bass_exec_p.bind(
            *operands,
            out_avals=tuple(out_avals),
            in_names=tuple(all_in_names),
            out_names=tuple(out_names),
            lowering_input_output_aliases=(),
            sim_require_finite=True,
            sim_require_nnan=True,
            nc=nc,
        )
        return tuple(outs)

    devices = jax.devices()[:N_CORES]
    mesh = Mesh(np.asarray(devices), ("core",))
    n_outs = len(out_names)
    in_specs = (PartitionSpec("core"),) * (n_params + n_outs)
    out_specs = (PartitionSpec("core"),) * n_outs
    sharded = jax.jit(
        shard_map(_body, mesh=mesh, in_specs=in_specs, out_specs=out_specs,
                  check_rep=False),
        keep_unused=True,
    )
    sharding = jax.sharding.NamedSharding(mesh, PartitionSpec("core"))
    dev_zeros = [
        jax.device_put(
            np.zeros((N_CORES * z.shape[0], *z.shape[1:]), z.dtype), sharding
        )
        for z in zero_outs
    ]

    def run(in_maps):
        concat_in = [
            np.concatenate(
                [np.asarray(in_maps[c][nm]) for c in range(N_CORES)], axis=0
            )
            for nm in in_names
        ]
        dev_in = [jax.device_put(a, sharding) for a in concat_in]
        outs = sharded(*dev_in, *dev_zeros)
        jax.block_until_ready(outs)
        return {
            nm: np.asarray(outs[i]) for i, nm in enumerate(out_names)
        }

    _RUNNER = run
    return _RUNNER


def kernel(
    encoder_out,
    decoder_hidden,
    W_enc,
    b_enc,
    W_dec,
    b_dec,
    w_full,
    b_full,
):
    enc = np.ascontiguousarray(np.asarray(encoder_out, dtype=np.float32))
    dec = np.ascontiguousarray(np.asarray(decoder_hidden, dtype=np.float32))
    wenc = np.ascontiguousarray(np.asarray(W_enc, dtype=np.float32))
    wdec = np.ascontiguousarray(np.asarray(W_dec, dtype=np.float32))
    benc = np.ascontiguousarray(np.asarray(b_enc, dtype=np.float32))
    bdec = np.ascontiguousarray(np.asarray(b_dec, dtype=np.float32))
    wfull = np.ascontiguousarray(np.asarray(w_full, dtype=np.float32))

    in_maps = []
    for c in range(N_CORES):
        in_maps.append(
            {
                "enc": enc[c * BL : (c + 1) * BL],
                "dec": dec[c * BL : (c + 1) * BL],
                "wenc": wenc,
                "wdec": wdec,
                "benc": benc,
                "bdec": bdec,
                "wfull": wfull,
                "ident": _IDENT,
            }
        )
    run = _get_runner()
    res = run(in_maps)
    context = res["context"].reshape(B, E)
    alpha = res["alpha"].reshape(B, L)
    return (context, alpha)
